# revision 30
# baseline (speedup 1.0000x reference)
"""Trainium2 Bass kernel for nn_EnsembleClustering_62646392979777.

Strategy (validated against the reference by a numpy prototype):
  * The full-resolution projection y = proj(x) is only ever consumed through
    spatial average-pools (7x7 agents, 2x2 clusters), and pooling commutes
    with the 1x1 conv.  So pool x first (56x56 -> 14x14 -> {7x7, 2x2}),
    then project the 53 pooled positions instead of 3136.  This removes
    ~98% of the FLOPs; the kernel becomes HBM-bound on reading x (308 MB)
    and writing the output (77 MB).
  * proj2 and the bilinear upsample also commute: run proj2 on the 7x7
    grid, then upsample as a dense [49 x 3136] matmul (exact linear op).
  * Data-parallel over batch: 16 batches -> 8 cores x 2.

Layout rules honored throughout: compute-engine partition bases are
32-aligned (BIR-verifier enforced); matmul operand bases in {0,32,64};
per-slice work is stacked along the free dimension.

Measured on 8 axon-tunneled TRN2 NeuronCores:
  relative error vs reference: 3.3e-7 (pure fp32 end to end)
  steady-state per-iteration:  ~230 us  (DMA roofline ~54 us/core:
  9.6 MB x-read + 9.6 MB y-write at ~358 GB/s)
"""
import sys
import numpy as np

sys.path.insert(0, "/opt/trn_rl_repo")

import concourse.bass as bass  # noqa: E402
import concourse.tile as tile  # noqa: E402
from concourse import bacc, mybir  # noqa: E402
from concourse.bass_utils import run_bass_kernel_spmd  # noqa: E402
from concourse.masks import make_identity  # noqa: E402

F32 = mybir.dt.float32
F32R = mybir.dt.float32r
AX = mybir.AxisListType
AF = mybir.ActivationFunctionType
OP = mybir.AluOpType

EPS = 1e-6
INV_SQRT_C = float(1.0 / np.sqrt(np.float32(48.0)))

_CACHE = {}


def _upsample_matrix():
    # jax.image.resize 'linear' 7->56 upsample: half-pixel centers, edge clamp
    U = np.zeros((56, 7), dtype=np.float64)
    for o in range(56):
        src = (o + 0.5) / 8.0 - 0.5
        i0 = int(np.floor(src))
        t = src - i0
        U[o, min(max(i0, 0), 6)] += 1.0 - t
        U[o, min(max(i0 + 1, 0), 6)] += t
    U = U.astype(np.float32)
    return np.einsum("Oi,Pj->ijOP", U, U).reshape(49, 3136).copy()


def build_nc(reps=1, stage="full"):
    # stage: "full" | "pool" (DMA-in + pooling only) | "noup" (skip upsample)
    nc = bacc.Bacc("TRN2", target_bir_lowering=False, debug=False,
                   enable_asserts=False)

    x_d = nc.dram_tensor("x", [2, 384, 3136], F32, kind="ExternalInput").ap()
    pwt_d = nc.dram_tensor("pwt", [128, 3, 1920], F32, kind="ExternalInput").ap()
    pbn_d = nc.dram_tensor("pbn", [48, 24], F32, kind="ExternalInput").ap()
    pbv_d = nc.dram_tensor("pbv", [1, 768], F32, kind="ExternalInput").ap()
    al_d = nc.dram_tensor("alph", [1, 64], F32, kind="ExternalInput").ap()
    be_d = nc.dram_tensor("beta", [1, 64], F32, kind="ExternalInput").ap()
    p2w_d = nc.dram_tensor("p2w", [48, 8, 384], F32, kind="ExternalInput").ap()
    p2b_d = nc.dram_tensor("p2b", [128, 3], F32, kind="ExternalInput").ap()
    mup_d = nc.dram_tensor("mup", [49, 3136], F32, kind="ExternalInput").ap()
    y_d = nc.dram_tensor("y", [2, 384, 3136], F32, kind="ExternalOutput").ap()

    with tile.TileContext(nc) as tc:
        with tc.tile_pool(name="w", bufs=1) as wp, \
             tc.tile_pool(name="xin", bufs=3) as xin, \
             tc.tile_pool(name="pool", bufs=2) as pp, \
             tc.tile_pool(name="st", bufs=1) as st, \
             tc.tile_pool(name="out", bufs=2) as outp, \
             tc.tile_pool(name="ps", bufs=2, space="PSUM") as ps:

            # ---------------- constants & weights ----------------
            ident = wp.tile([128, 128], F32, tag="ident")
            make_identity(nc, ident[:])
            ones_c = wp.tile([49, 1], F32, tag="ones_c")   # column of ones
            nc.vector.memset(ones_c[:], 1.0)
            ones_r = wp.tile([1, 768], F32, tag="ones_r")  # row of ones
            nc.vector.memset(ones_r[:], 1.0)

            PWT = wp.tile([128, 3, 1920], F32, tag="pwt")
            nc.sync.dma_start(PWT[:], pwt_d)
            P2W = wp.tile([48, 8, 384], F32, tag="p2w")
            nc.sync.dma_start(P2W[:], p2w_d)
            MUP = wp.tile([49, 3136], F32, tag="mup")
            nc.sync.dma_start(MUP[:], mup_d)
            PBN = wp.tile([48, 24], F32, tag="pbn")
            nc.sync.dma_start(PBN[:], pbn_d)
            PBV1 = wp.tile([1, 768], F32, tag="pbv1")
            nc.sync.dma_start(PBV1[:], pbv_d)
            P2B = wp.tile([128, 3], F32, tag="p2b")
            nc.sync.dma_start(P2B[:], p2b_d)
            AL1 = wp.tile([1, 64], F32, tag="al1")
            nc.sync.dma_start(AL1[:], al_d)
            BE1 = wp.tile([1, 64], F32, tag="be1")
            nc.sync.dma_start(BE1[:], be_d)

            # broadcast alpha/beta/bias_v across partitions via K=1 matmuls
            ALB = wp.tile([49, 64], F32, tag="alb")
            BEB = wp.tile([49, 64], F32, tag="beb")
            for src, dst in ((AL1, ALB), (BE1, BEB)):
                pt = ps.tile([49, 64], F32, tag="B")
                nc.tensor.matmul(pt[:], ones_r[:, :49], src[:], start=True, stop=True)
                nc.vector.tensor_copy(dst[:], pt[:])
            PB49 = wp.tile([49, 768], F32, tag="pb49")
            PB4 = wp.tile([4, 768], F32, tag="pb4")
            for half in range(2):
                pt = ps.tile([49, 384], F32, tag="B")
                nc.tensor.matmul(pt[:], ones_r[:, :49], PBV1[:, 384 * half:384 * (half + 1)],
                                 start=True, stop=True)
                nc.vector.tensor_copy(PB49[:, 384 * half:384 * (half + 1)], pt[:])
                pt2 = ps.tile([4, 384], F32, tag="B")
                nc.tensor.matmul(pt2[:], ones_r[:, :4], PBV1[:, 384 * half:384 * (half + 1)],
                                 start=True, stop=True)
                nc.vector.tensor_copy(PB4[:, 384 * half:384 * (half + 1)], pt2[:])

            # ---------------- per-batch pipeline ----------------
            # reps>1 re-emits the body for steady-state benchmarking
            for rep_bi in range(2 * reps):
                bi = rep_bi % 2
                # ---- Stage A: load & pool x ----
                XP = st.tile([128, 3, 53], F32, tag=f"xp{bi}")
                for j in range(3):
                    X = xin.tile([128, 3136], F32, tag="x")
                    nc.sync.dma_start(X[:], x_d[bi, 128 * j:128 * (j + 1), :])
                    R1 = pp.tile([128, 784], F32, tag="r1")
                    nc.vector.reduce_sum(
                        R1[:], X[:].rearrange("p (a b) -> p a b", b=4), axis=AX.X)
                    R2 = pp.tile([128, 196], F32, tag="r2")
                    nc.vector.reduce_sum(
                        R2[:],
                        R1[:].rearrange("p (oh hi w) -> p oh w hi", oh=14, hi=4, w=14),
                        axis=AX.X)
                    P7 = pp.tile([128, 49], F32, tag="p7")
                    nc.vector.reduce_sum(
                        P7[:],
                        R2[:].rearrange("p (oh hi ow wi) -> p oh ow hi wi",
                                        oh=7, hi=2, ow=7, wi=2),
                        axis=AX.XY)
                    P2t = pp.tile([128, 4], F32, tag="p2")
                    nc.vector.reduce_sum(
                        P2t[:],
                        R2[:].rearrange("p (oh hi ow wi) -> p oh ow hi wi",
                                        oh=2, hi=7, ow=2, wi=7),
                        axis=AX.XY)
                    nc.vector.tensor_scalar_mul(XP[:, j, 0:49], P7[:], 1.0 / 64.0)
                    nc.vector.tensor_scalar_mul(XP[:, j, 49:53], P2t[:], 1.0 / 784.0)

                if stage == "pool":
                    ri = (rep_bi // 2) % 19
                    nc.sync.dma_start(
                        y_d[bi, 0:128, 159 * ri:159 * (ri + 1)],
                        XP[:].rearrange("p a b -> p (a b)"))
                    continue

                # ---- Stage B1: natural-layout projection (groups p, k0, k1) ----
                # chunk t = g*8 + h ; o-range = G[g]*384 + 48h,  G = [0, 1, 3]
                Ysb = st.tile([48, 24, 53], F32, tag=f"ysb{bi}")
                for g, G in enumerate((0, 1, 3)):
                    for h in range(8):
                        t = g * 8 + h
                        o0 = G * 384 + 48 * h
                        pt = ps.tile([48, 53], F32, tag="A")
                        for j in range(3):
                            nc.tensor.matmul(pt[:], PWT[:, j, o0:o0 + 48], XP[:, j, :],
                                             start=(j == 0), stop=(j == 2))
                        nc.scalar.activation(Ysb[:, t, :], pt[:], AF.Identity,
                                             bias=PBN[:, t:t + 1], scale=1.0)

                # ---- Stage B2: transposed projection of v-groups ----
                # V7T[n, i*384 + hc] = y7[(2+2i)*384 + hc, n];  VCt likewise for y2
                V7T = st.tile([49, 768], F32, tag=f"v7t{bi}")
                VCt = st.tile([4, 768], F32, tag=f"vct{bi}")
                for i in range(2):
                    o0 = (2 + 2 * i) * 384
                    pt = ps.tile([49, 384], F32, tag="B")
                    for j in range(3):
                        nc.tensor.matmul(pt[:], XP[:, j, 0:49], PWT[:, j, o0:o0 + 384],
                                         start=(j == 0), stop=(j == 2))
                    nc.vector.tensor_tensor(V7T[:, 384 * i:384 * (i + 1)], pt[:],
                                            PB49[:, 384 * i:384 * (i + 1)], OP.add)
                    pt2 = ps.tile([4, 384], F32, tag="B")
                    for j in range(3):
                        nc.tensor.matmul(pt2[:], XP[:, j, 49:53], PWT[:, j, o0:o0 + 384],
                                         start=(j == 0), stop=(j == 2))
                    nc.vector.tensor_tensor(VCt[:, 384 * i:384 * (i + 1)], pt2[:],
                                            PB4[:, 384 * i:384 * (i + 1)], OP.add)

                # ---- Stage C: attention / clustering (8 slices, free-stacked) ----
                S0f = st.tile([4, 8, 49], F32, tag=f"s0{bi}")
                S1f = st.tile([49, 8, 4], F32, tag=f"s1{bi}")
                for h in range(8):
                    k0 = Ysb[:, 8 + h, 0:49]
                    kc0 = Ysb[:, 8 + h, 49:53]
                    k1 = Ysb[:, 16 + h, 0:49]
                    kc1 = Ysb[:, 16 + h, 49:53]
                    pt = ps.tile([4, 49], F32, tag="C")
                    nc.tensor.matmul(pt[:], kc0, k0, start=True, stop=True)
                    nc.vector.tensor_scalar_mul(S0f[:, h, :], pt[:], INV_SQRT_C)
                    pt2 = ps.tile([49, 4], F32, tag="C")
                    nc.tensor.matmul(pt2[:], k1, kc1, start=True, stop=True)
                    nc.scalar.mul(S1f[:, h, :], pt2[:], INV_SQRT_C)

                # softmax0 over n (module 0): S0f [4, (8,49)]
                M0 = st.tile([4, 8], F32, tag=f"m0{bi}")
                nc.vector.reduce_max(M0[:], S0f[:], axis=AX.X, negate=True)
                E0 = st.tile([4, 8, 49], F32, tag=f"e0{bi}")
                nc.vector.tensor_tensor(E0[:], S0f[:],
                                        M0[:, :, None].to_broadcast((4, 8, 49)), OP.add)
                nc.scalar.activation(E0[:], E0[:], AF.Exp)
                SM0 = st.tile([4, 8], F32, tag=f"sm0{bi}")
                nc.vector.reduce_sum(SM0[:], E0[:], axis=AX.X)
                nc.vector.reciprocal(SM0[:], SM0[:])
                A0 = st.tile([4, 8, 49], F32, tag=f"a0{bi}")
                nc.vector.tensor_tensor(A0[:], E0[:],
                                        SM0[:, :, None].to_broadcast((4, 8, 49)), OP.mult)

                # softmax1 over clusters (module 1): S1f [49, (8,4)]
                M1 = st.tile([49, 8], F32, tag=f"m1{bi}")
                nc.vector.reduce_max(M1[:], S1f[:], axis=AX.X, negate=True)
                E1 = st.tile([49, 8, 4], F32, tag=f"e1{bi}")
                nc.vector.tensor_tensor(E1[:], S1f[:],
                                        M1[:, :, None].to_broadcast((49, 8, 4)), OP.add)
                nc.scalar.activation(E1[:], E1[:], AF.Exp)
                SM1 = st.tile([49, 8], F32, tag=f"sm1{bi}")
                nc.vector.reduce_sum(SM1[:], E1[:], axis=AX.X)
                nc.vector.reciprocal(SM1[:], SM1[:])
                A1T = st.tile([49, 8, 4], F32, tag=f"a1t{bi}")
                nc.vector.tensor_tensor(A1T[:], E1[:],
                                        SM1[:, :, None].to_broadcast((49, 8, 4)), OP.mult)

                # transpose module-0 attention: A0 [4,49] slices -> A0T [49,(8,4)]
                A0T = st.tile([49, 8, 4], F32, tag=f"a0t{bi}")
                for h in range(8):
                    pt = ps.tile([49, 4], F32, tag="B")
                    nc.tensor.transpose(pt[:], A0[:, h, :], ident[:4, :4])
                    nc.vector.tensor_copy(A0T[:, h, :], pt[:])

                # fuzzy-membership normalizer: 1/(sum_n memb + eps), PE-broadcast
                ptd = ps.tile([1, 32], F32, tag="C")
                nc.tensor.matmul(ptd[:], ones_c[:], A1T[:].rearrange("p a b -> p (a b)"),
                                 start=True, stop=True)
                DE = st.tile([1, 32], F32, tag=f"de{bi}")
                nc.vector.tensor_scalar_add(DE[:], ptd[:], EPS)
                nc.vector.reciprocal(DE[:], DE[:])
                ptb = ps.tile([49, 32], F32, tag="C")
                nc.tensor.matmul(ptb[:], ones_r[:, :49], DE[:], start=True, stop=True)
                A1N = st.tile([49, 8, 4], F32, tag=f"a1n{bi}")
                nc.vector.tensor_tensor(A1N[:].rearrange("p a b -> p (a b)"),
                                        A1T[:].rearrange("p a b -> p (a b)"),
                                        ptb[:], OP.mult)

                # agg = attn @ v (+ vc), stacked [4, (2, 8, 48)]
                AGGf = st.tile([4, 2, 8, 48], F32, tag=f"aggf{bi}")
                for i in range(2):
                    AT = A0T if i == 0 else A1N
                    for h in range(8):
                        pt = ps.tile([4, 48], F32, tag="A")
                        nc.tensor.matmul(pt[:], AT[:, h, :],
                                         V7T[:, 384 * i + 48 * h:384 * i + 48 * (h + 1)],
                                         start=True, stop=True)
                        nc.vector.tensor_tensor(
                            AGGf[:, i, h, :], pt[:],
                            VCt[:, 384 * i + 48 * h:384 * i + 48 * (h + 1)], OP.add)

                # agg row norms -> normalized AGGN
                SQ = st.tile([4, 768], F32, tag=f"sq{bi}")
                nc.scalar.activation(SQ[:], AGGf[:].rearrange("p a b c -> p (a b c)"),
                                     AF.Square)
                SS = st.tile([4, 16], F32, tag=f"ss{bi}")
                nc.vector.reduce_sum(SS[:], SQ[:].rearrange("p (g c) -> p g c", c=48),
                                     axis=AX.X)
                nc.scalar.activation(SS[:], SS[:], AF.Sqrt)
                nc.vector.tensor_scalar_add(SS[:], SS[:], EPS)
                nc.vector.reciprocal(SS[:], SS[:])
                AGGN = st.tile([4, 2, 8, 48], F32, tag=f"aggn{bi}")
                nc.vector.tensor_tensor(
                    AGGN[:].rearrange("p a b c -> p (a b) c"),
                    AGGf[:].rearrange("p a b c -> p (a b) c"),
                    SS[:, :, None].to_broadcast((4, 16, 48)), OP.mult)

                # transpose AGGN slices -> AGGNT [48, (8 slices, 8 m)]
                AGGNT = st.tile([48, 8, 8], F32, tag=f"aggnt{bi}")
                for i in range(2):
                    for h in range(8):
                        pt = ps.tile([48, 4], F32, tag="B")
                        nc.tensor.transpose(pt[:], AGGN[:, i, h, :], ident[:4, :4])
                        nc.vector.tensor_copy(AGGNT[:, h, 4 * i:4 * (i + 1)], pt[:])

                # p-token norms: ||p||^2 over c via ones-matmul
                SQP = st.tile([48, 8, 49], F32, tag=f"sqp{bi}")
                nc.scalar.activation(SQP[:], Ysb[:, 0:8, 0:49], AF.Square)
                ptn = ps.tile([49, 8], F32, tag="C")
                for h in range(8):
                    nc.tensor.matmul(ptn[:, h:h + 1], SQP[:, h, :], ones_c[:48, :],
                                     start=True, stop=True)
                RP = st.tile([49, 8], F32, tag=f"rp{bi}")
                nc.scalar.activation(RP[:], ptn[:], AF.Sqrt)
                nc.vector.tensor_scalar_add(RP[:], RP[:], EPS)
                nc.vector.reciprocal(RP[:], RP[:])

                # sim^T [49, (8 slices, 8 m)] = (p^T @ aggn^T) * rp, then alpha/beta
                ptm = ps.tile([49, 64], F32, tag="C")
                for h in range(8):
                    nc.tensor.matmul(ptm[:, 8 * h:8 * (h + 1)], Ysb[:, h, 0:49],
                                     AGGNT[:, h, :], start=True, stop=True)
                SIMT = st.tile([49, 8, 8], F32, tag=f"simt{bi}")
                nc.vector.tensor_tensor(SIMT[:], ptm[:].rearrange("p (a b) -> p a b", b=8),
                                        RP[:, :, None].to_broadcast((49, 8, 8)), OP.mult)
                nc.vector.tensor_tensor(SIMT[:].rearrange("p a b -> p (a b)"),
                                        SIMT[:].rearrange("p a b -> p (a b)"),
                                        ALB[:], OP.mult)
                nc.vector.tensor_tensor(SIMT[:].rearrange("p a b -> p (a b)"),
                                        SIMT[:].rearrange("p a b -> p (a b)"),
                                        BEB[:], OP.add)

                # assignment softmax over the 8 clusters (inner free dim)
                MM = st.tile([49, 8], F32, tag=f"mm{bi}")
                nc.vector.reduce_max(MM[:], SIMT[:], axis=AX.X, negate=True)
                EX = st.tile([49, 8, 8], F32, tag=f"ex{bi}")
                nc.vector.tensor_tensor(EX[:], SIMT[:],
                                        MM[:, :, None].to_broadcast((49, 8, 8)), OP.add)
                nc.scalar.activation(EX[:], EX[:], AF.Exp)
                SMS = st.tile([49, 8], F32, tag=f"sms{bi}")
                nc.vector.reduce_sum(SMS[:], EX[:], axis=AX.X)
                nc.vector.reciprocal(SMS[:], SMS[:])
                ASGT = st.tile([49, 8, 8], F32, tag=f"asgt{bi}")
                nc.vector.tensor_tensor(ASGT[:], EX[:],
                                        SMS[:, :, None].to_broadcast((49, 8, 8)), OP.mult)

                # transpose assignment to m-on-partitions (two 4-row tiles)
                ASG0 = st.tile([4, 8, 49], F32, tag=f"asg0{bi}")
                ASG1 = st.tile([4, 8, 49], F32, tag=f"asg1{bi}")
                for h in range(8):
                    pt = ps.tile([4, 49], F32, tag="B")
                    nc.tensor.transpose(pt[:], ASGT[:, h, 0:4], ident[:49, :49])
                    nc.vector.tensor_copy(ASG0[:, h, :], pt[:])
                    pt2 = ps.tile([4, 49], F32, tag="B")
                    nc.tensor.transpose(pt2[:], ASGT[:, h, 4:8], ident[:49, :49])
                    nc.vector.tensor_copy(ASG1[:, h, :], pt2[:])

                # out_low per slice: z [48, 49] = agg^T @ assignment
                Zf = st.tile([48, 8, 49], F32, tag=f"zf{bi}")
                for h in range(8):
                    pt = ps.tile([48, 49], F32, tag="A")
                    nc.tensor.matmul(pt[:], AGGf[:, 0, h, :], ASG0[:, h, :],
                                     start=True, stop=False)
                    nc.tensor.matmul(pt[:], AGGf[:, 1, h, :], ASG1[:, h, :],
                                     start=False, stop=True)
                    nc.vector.tensor_copy(Zf[:, h, :], pt[:])

                # proj2 on the 7x7 grid (contract d in 8 head-chunks of 48)
                Z2 = st.tile([128, 3, 49], F32, tag=f"z2{bi}")
                for oi in range(3):
                    pt = ps.tile([128, 49], F32, tag="A")
                    for h in range(8):
                        nc.tensor.matmul(pt[:], P2W[:, h, 128 * oi:128 * (oi + 1)],
                                         Zf[:, h, :], start=(h == 0), stop=(h == 7))
                    nc.scalar.activation(Z2[:, oi, :], pt[:], AF.Identity,
                                         bias=P2B[:, oi:oi + 1], scale=1.0)

                # transpose z2 -> [49, 384] for the upsample matmul
                Z2T = st.tile([49, 384], F32, tag=f"z2t{bi}")
                for oi in range(3):
                    pt = ps.tile([49, 128], F32, tag="B")
                    nc.tensor.transpose(pt[:], Z2[:, oi, :], ident[:])
                    nc.vector.tensor_copy(Z2T[:, 128 * oi:128 * (oi + 1)], pt[:])

                if stage == "noup":
                    ri = (rep_bi // 2) % 8
                    nc.sync.dma_start(y_d[bi, 0:49, 384 * ri:384 * (ri + 1)], Z2T[:])
                    continue

                # upsample: out[128, 3136] = z2T^T @ MUP, tile N by 448
                for oi in range(3):
                    OUT = outp.tile([128, 3136], F32, tag="out")
                    for nt in range(7):
                        pt = ps.tile([128, 448], F32, tag="U")
                        nc.tensor.matmul(pt[:],
                                         Z2T[:, 128 * oi:128 * (oi + 1)],
                                         MUP[:, 448 * nt:448 * (nt + 1)],
                                         start=True, stop=True)
                        eng = nc.vector if nt % 2 == 0 else nc.scalar
                        if eng is nc.vector:
                            nc.vector.tensor_copy(OUT[:, 448 * nt:448 * (nt + 1)], pt[:])
                        else:
                            nc.scalar.copy(OUT[:, 448 * nt:448 * (nt + 1)], pt[:])
                    nc.sync.dma_start(y_d[bi, 128 * oi:128 * (oi + 1), :], OUT[:])

    nc.compile()
    return nc


def _prep_weights(proj_w, proj_b, sim_alpha, sim_beta, proj2_w, proj2_b):
    pwT = np.ascontiguousarray(proj_w.T)                       # [384, 1920]
    pwt = np.ascontiguousarray(pwT.reshape(3, 128, 1920).transpose(1, 0, 2))
    G = (0, 1, 3)
    pbn = np.empty((48, 24), np.float32)
    for t in range(24):
        g, h = divmod(t, 8)
        o0 = G[g] * 384 + 48 * h
        pbn[:, t] = proj_b[o0:o0 + 48]
    pbv = np.concatenate([proj_b[768:1152], proj_b[1536:1920]])[None, :]
    alph = np.tile(sim_alpha, 8)[None, :].astype(np.float32)
    beta = np.tile(sim_beta, 8)[None, :].astype(np.float32)
    p2wT = np.ascontiguousarray(proj2_w.T)                     # [384, 384]
    p2w = np.ascontiguousarray(p2wT.reshape(8, 48, 384).transpose(1, 0, 2))
    p2b = np.ascontiguousarray(proj2_b.reshape(3, 128).T)
    mup = _upsample_matrix()
    return {"pwt": pwt, "pbn": pbn,
            "pbv": np.ascontiguousarray(pbv), "alph": alph, "beta": beta,
            "p2w": p2w, "p2b": p2b, "mup": mup}


def kernel(x, proj_w, proj_b, sim_alpha, sim_beta, proj2_w, proj2_b):
    x = np.asarray(x, np.float32)
    proj_w = np.asarray(proj_w, np.float32)
    proj_b = np.asarray(proj_b, np.float32)
    sim_alpha = np.asarray(sim_alpha, np.float32)
    sim_beta = np.asarray(sim_beta, np.float32)
    proj2_w = np.asarray(proj2_w, np.float32)
    proj2_b = np.asarray(proj2_b, np.float32)

    if "nc" not in _CACHE:
        _CACHE["nc"] = build_nc()
    nc = _CACHE["nc"]

    w = _prep_weights(proj_w, proj_b, sim_alpha, sim_beta, proj2_w, proj2_b)
    B = x.shape[0]
    xr = x.reshape(8, B // 8, 384, 3136)
    in_maps = [dict(w, x=np.ascontiguousarray(xr[c])) for c in range(8)]

    res = run_bass_kernel_spmd(nc, in_maps, core_ids=list(range(8)))
    out = np.concatenate([r["y"] for r in res.results], axis=0)
    return out.reshape(16, 384, 56, 56).astype(np.float32, copy=False)


if __name__ == "__main__":
    rng = np.random.default_rng(0)
    inputs = {
        "x": rng.standard_normal((16, 384, 56, 56), dtype=np.float32),
        "proj_w": rng.standard_normal((1920, 384), dtype=np.float32) * 384 ** -0.5,
        "proj_b": np.zeros(1920, np.float32),
        "sim_alpha": np.ones(8, np.float32),
        "sim_beta": np.zeros(8, np.float32),
        "proj2_w": rng.standard_normal((384, 384), dtype=np.float32) * 384 ** -0.5,
        "proj2_b": np.zeros(384, np.float32),
    }
    out = kernel(**inputs)
    print("kernel ran, output", out.shape, out.dtype, float(np.abs(out).max()))
